# revision 6
# baseline (speedup 1.0000x reference)
"""Trainium2 Bass kernel for DecouplingSpecificSpecificLoss.

Reference computation: reshape [16384, 2048] -> [4096 chunks, 4 views, 2048],
L2-normalize rows, per-chunk 4x4 cosine-similarity matrix, clip to
[5e-4, 0.9995], loss = sum over chunks of mean(-log(1 - sim)).

Strategy (8 NeuronCores, data parallel over chunks):
  - Each core gets 2048 contiguous rows (512 chunks of 4 rows).
  - Layout: one chunk per SBUF partition -> tiles of [128 partitions, 4*2048].
    Each partition's 32 KiB is contiguous in HBM, so DMA runs at line rate.
  - Per tile the only heavy math is 10 length-2048 dot products per chunk:
      * 4 self-dots (squared norms): ScalarE activation(Square, accum_out)
      * 4 cross-dots on VectorE via fused tensor_tensor_reduce
      * 2 cross-dots on GpSimd via scalar_tensor_tensor(accum_out)
    All engines stream the f32 data once -> every engine sits below the
    ~47us/core HBM roofline for the 16 MiB shard.
  - Tiny [128, 10] result per tile is DMA'd out; the host finishes the
    normalize/clip/log/reduce on 40960 scalars. The diagonal of each 4x4
    sim matrix is always 1 -> clips to 0.9995, so it contributes a closed
    form constant and is never computed on device.
"""

import json
import sys

if "/opt/trn_rl_repo" not in sys.path:
    sys.path.insert(0, "/opt/trn_rl_repo")

import numpy as np

import concourse.bass as bass
import concourse.mybir as mybir
import concourse.tile as tile
from concourse.bass_utils import run_bass_kernel_spmd

N_CORES = 8
B, D = 16384, 2048
V = 4                                  # views (rows) per chunk
ROWS_PER_CORE = B // N_CORES           # 2048
CHUNKS_PER_CORE = ROWS_PER_CORE // V   # 512
P = 128                                # SBUF partitions
TILES = CHUNKS_PER_CORE // P           # 4
FREE = V * D                           # 8192 f32 per partition

CLAMP_MIN = 0.0005
CLAMP_MAX = 0.9995
NORM_EPS = 1e-12

# (view_i, view_j) cross pairs; all six run on VectorE in bf16.
ALL_PAIRS = [(0, 1), (0, 2), (0, 3), (1, 2), (1, 3), (2, 3)]


def build_bass():
    f32 = mybir.dt.float32
    nc = bass.Bass()
    x = nc.declare_dram_parameter("x", [ROWS_PER_CORE, D], f32, isOutput=False)
    out = nc.declare_dram_parameter("out", [TILES, P, 10], f32, isOutput=True)

    bf16 = mybir.dt.bfloat16
    with tile.TileContext(nc) as tc:
        with (
            tc.tile_pool(name="xin", bufs=3) as xin_pool,
            tc.tile_pool(name="xb", bufs=2) as xb_pool,
            tc.tile_pool(name="sdve", bufs=2) as sdve_pool,
            tc.tile_pool(name="sact", bufs=2) as sact_pool,
            tc.tile_pool(name="acc", bufs=4) as acc_pool,
        ):
            for t in range(TILES):
                xt = xin_pool.tile([P, FREE], f32, tag="xt")
                src = x[512 * t : 512 * (t + 1), :].rearrange(
                    "(p r) d -> p (r d)", r=V
                )
                nc.sync.dma_start(xt[:], src)

                views = [xt[:, D * v : D * (v + 1)] for v in range(V)]
                xb = xb_pool.tile([P, FREE], bf16, tag="xb")
                bviews = [xb[:, D * v : D * (v + 1)] for v in range(V)]
                dots = acc_pool.tile([P, len(ALL_PAIRS)], f32, tag="dots")
                norms2 = acc_pool.tile([P, V], f32, tag="n2")

                # squared norms in f32 on ScalarE (fused square + accumulate)
                for v in range(V):
                    s = sact_pool.tile([P, D], f32, tag="sact")
                    nc.scalar.activation(
                        s[:],
                        views[v],
                        mybir.ActivationFunctionType.Square,
                        accum_out=norms2[:, v : v + 1],
                    )

                # f32 -> bf16 casts: view 0 on ScalarE, views 1-3 on VectorE
                nc.scalar.activation(
                    bviews[0], views[0], mybir.ActivationFunctionType.Copy
                )
                for v in range(1, V):
                    nc.vector.tensor_copy(bviews[v], views[v])

                # six cross-dots on VectorE: fused multiply + free-axis accum
                for k, (a, b) in enumerate(ALL_PAIRS):
                    s = sdve_pool.tile([P, D], bf16, tag="sdve")
                    nc.vector.scalar_tensor_tensor(
                        out=s[:],
                        in0=bviews[a],
                        scalar=0.0,
                        in1=bviews[b],
                        op0=mybir.AluOpType.bypass,
                        op1=mybir.AluOpType.mult,
                        accum_out=dots[:, k : k + 1],
                    )

                nc.sync.dma_start(out[t, :, 0:6], dots[:])
                nc.sync.dma_start(out[t, :, 6:10], norms2[:])

    return nc


def _split_multiwait_bir(bir_json: bytes) -> bytes:
    """Legalize BIR for this walrus build: it rejects instructions carrying
    more than one semaphore wait ("Too many sync wait commands"). Tile emits
    multi-wait instructions (the tail Drain waits on every live sem; compute
    ops can wait on several producers). Hoist all but one wait onto fresh
    standalone EventSemaphore instructions inserted just before the original
    on the same engine — the engine sequencer executes them in order, so the
    semantics are unchanged.
    """
    mod = json.loads(bir_json)
    n_new = 0
    for fn in mod["functions"]:
        for bb in fn["blocks"]:
            out_insts = []
            for inst in bb["instructions"]:
                si = inst.get("sync_info") or {}
                waits = si.get("on_wait") or []
                cap = 2 if inst.get("opcode") == "EventSemaphore" else 1
                if len(waits) > cap:
                    keep = waits[: cap - 1] if cap > 1 else []
                    hoist = waits[len(keep) : -1]
                    last = [waits[-1]]
                    for w in hoist:
                        n_new += 1
                        out_insts.append(
                            {
                                "debug": inst.get("debug", 0),
                                "engine": inst["engine"],
                                "ins": [],
                                "name": f"{inst['name']}-hw{n_new}",
                                "opcode": "EventSemaphore",
                                "outs": [],
                                "sync_info": {"on_update": [], "on_wait": [w]},
                            }
                        )
                    si["on_wait"] = keep + last
                out_insts.append(inst)
            bb["instructions"] = out_insts
    return json.dumps(mod).encode()


_NC_CACHE = None


def _get_nc():
    global _NC_CACHE
    if _NC_CACHE is None:
        nc = build_bass()
        fixed = _split_multiwait_bir(nc.to_json_bytes())
        nc.to_json_bytes = lambda: fixed
        _NC_CACHE = nc
    return _NC_CACHE


def run(specific_features, trace=False, **trace_kw):
    """Run the device kernel; returns (per-core raw outputs, BassKernelResults)."""
    xs = np.asarray(specific_features, dtype=np.float32)
    assert xs.shape == (B, D), xs.shape
    shards = [
        np.ascontiguousarray(xs[c * ROWS_PER_CORE : (c + 1) * ROWS_PER_CORE])
        for c in range(N_CORES)
    ]
    in_maps = [{"x": s} for s in shards]
    nc = _get_nc()
    res = run_bass_kernel_spmd(
        nc, in_maps, list(range(N_CORES)), trace=trace, **trace_kw
    )
    outs = [r["out"] for r in res.results]
    return outs, res


def postprocess(outs):
    """Finish the loss from per-core [TILES, P, 10] raw dot/norm tensors."""
    # f32-faithful diagonal term: sim_ii == 1 always clips to CLAMP_MAX.
    diag_term = float(-np.log(np.float32(1.0) - np.float32(CLAMP_MAX)))
    total = 0.0
    ii = [p[0] for p in ALL_PAIRS]
    jj = [p[1] for p in ALL_PAIRS]
    for arr in outs:
        a = np.asarray(arr, dtype=np.float64)  # [TILES, P, 10]
        dots = a[..., 0:6]
        n2 = a[..., 6:10]
        norms = np.maximum(np.sqrt(n2), NORM_EPS)
        sim = dots / (norms[..., ii] * norms[..., jj])
        sim = np.clip(sim, CLAMP_MIN, CLAMP_MAX)
        # each unordered pair appears twice in the symmetric 4x4 matrix
        total += 2.0 * np.sum(-np.log1p(-sim))
    total += (B // V) * V * diag_term
    return np.float32(total / (V * V))


def kernel(specific_features):
    outs, _ = run(specific_features, trace=False)
    return postprocess(outs)


if __name__ == "__main__":
    x = np.random.default_rng(0).standard_normal((B, D)).astype(np.float32)
    print(kernel(x))


# revision 7
# speedup vs baseline: 1.1705x; 1.1705x over previous
"""Trainium2 Bass kernel for DecouplingSpecificSpecificLoss.

Reference computation: reshape [16384, 2048] -> [4096 chunks, 4 views, 2048],
L2-normalize rows, per-chunk 4x4 cosine-similarity matrix, clip to
[5e-4, 0.9995], loss = sum over chunks of mean(-log(1 - sim)).

Strategy (8 NeuronCores, data parallel over chunks):
  - Each core gets 2048 contiguous rows (512 chunks of 4 rows).
  - Layout: one chunk per SBUF partition -> tiles of [128 partitions, 4*2048].
    Each partition's 32 KiB is contiguous in HBM, so DMA runs at line rate.
  - Per tile the only heavy math is 10 length-2048 dot products per chunk:
      * 4 self-dots (squared norms): ScalarE activation(Square, accum_out)
      * 4 cross-dots on VectorE via fused tensor_tensor_reduce
      * 2 cross-dots on GpSimd via scalar_tensor_tensor(accum_out)
    All engines stream the f32 data once -> every engine sits below the
    ~47us/core HBM roofline for the 16 MiB shard.
  - Tiny [128, 10] result per tile is DMA'd out; the host finishes the
    normalize/clip/log/reduce on 40960 scalars. The diagonal of each 4x4
    sim matrix is always 1 -> clips to 0.9995, so it contributes a closed
    form constant and is never computed on device.
"""

import json
import sys

if "/opt/trn_rl_repo" not in sys.path:
    sys.path.insert(0, "/opt/trn_rl_repo")

import numpy as np

import concourse.bass as bass
import concourse.mybir as mybir
import concourse.tile as tile
from concourse.bass_utils import run_bass_kernel_spmd

N_CORES = 8
B, D = 16384, 2048
V = 4                                  # views (rows) per chunk
ROWS_PER_CORE = B // N_CORES           # 2048
CHUNKS_PER_CORE = ROWS_PER_CORE // V   # 512
P = 128                                # SBUF partitions
TILES = CHUNKS_PER_CORE // P           # 4
FREE = V * D                           # 8192 f32 per partition

CLAMP_MIN = 0.0005
CLAMP_MAX = 0.9995
NORM_EPS = 1e-12

# (view_i, view_j) cross pairs; all six run on VectorE in bf16.
ALL_PAIRS = [(0, 1), (0, 2), (0, 3), (1, 2), (1, 3), (2, 3)]


def build_bass():
    f32 = mybir.dt.float32
    nc = bass.Bass()
    x = nc.declare_dram_parameter("x", [ROWS_PER_CORE, D], f32, isOutput=False)
    out = nc.declare_dram_parameter("out", [TILES, P, 10], f32, isOutput=True)

    with tile.TileContext(nc) as tc:
        with (
            tc.tile_pool(name="xin", bufs=TILES) as xin_pool,
            tc.tile_pool(name="sdve", bufs=2) as sdve_pool,
            tc.tile_pool(name="sact", bufs=2) as sact_pool,
            tc.tile_pool(name="acc", bufs=4) as acc_pool,
        ):
            for t in range(TILES):
                xt = xin_pool.tile([P, FREE], f32, tag="xt")
                src = x[512 * t : 512 * (t + 1), :].rearrange(
                    "(p r) d -> p (r d)", r=V
                )
                nc.sync.dma_start(xt[:], src)

                views = [xt[:, D * v : D * (v + 1)] for v in range(V)]
                dots = acc_pool.tile([P, len(ALL_PAIRS)], f32, tag="dots")
                norms2 = acc_pool.tile([P, V], f32, tag="n2")

                # squared norms on ScalarE (fused square + accumulate)
                for v in range(V):
                    s = sact_pool.tile([P, D], f32, tag="sact")
                    nc.scalar.activation(
                        s[:],
                        views[v],
                        mybir.ActivationFunctionType.Square,
                        accum_out=norms2[:, v : v + 1],
                    )

                # six cross-dots on VectorE: fused multiply + free-axis accum
                for k, (a, b) in enumerate(ALL_PAIRS):
                    s = sdve_pool.tile([P, D], f32, tag="sdve")
                    nc.vector.scalar_tensor_tensor(
                        out=s[:],
                        in0=views[a],
                        scalar=0.0,
                        in1=views[b],
                        op0=mybir.AluOpType.bypass,
                        op1=mybir.AluOpType.mult,
                        accum_out=dots[:, k : k + 1],
                    )

                nc.sync.dma_start(out[t, :, 0:6], dots[:])
                nc.sync.dma_start(out[t, :, 6:10], norms2[:])

    return nc


def _split_multiwait_bir(bir_json: bytes) -> bytes:
    """Legalize BIR for this walrus build: it rejects instructions carrying
    more than one semaphore wait ("Too many sync wait commands"). Tile emits
    multi-wait instructions (the tail Drain waits on every live sem; compute
    ops can wait on several producers). Hoist all but one wait onto fresh
    standalone EventSemaphore instructions inserted just before the original
    on the same engine — the engine sequencer executes them in order, so the
    semantics are unchanged.
    """
    mod = json.loads(bir_json)
    n_new = 0
    for fn in mod["functions"]:
        for bb in fn["blocks"]:
            out_insts = []
            for inst in bb["instructions"]:
                si = inst.get("sync_info") or {}
                waits = si.get("on_wait") or []
                cap = 2 if inst.get("opcode") == "EventSemaphore" else 1
                if len(waits) > cap:
                    keep = waits[: cap - 1] if cap > 1 else []
                    hoist = waits[len(keep) : -1]
                    last = [waits[-1]]
                    for w in hoist:
                        n_new += 1
                        out_insts.append(
                            {
                                "debug": inst.get("debug", 0),
                                "engine": inst["engine"],
                                "ins": [],
                                "name": f"{inst['name']}-hw{n_new}",
                                "opcode": "EventSemaphore",
                                "outs": [],
                                "sync_info": {"on_update": [], "on_wait": [w]},
                            }
                        )
                    si["on_wait"] = keep + last
                out_insts.append(inst)
            bb["instructions"] = out_insts
    return json.dumps(mod).encode()


_NC_CACHE = None


def _get_nc():
    global _NC_CACHE
    if _NC_CACHE is None:
        nc = build_bass()
        fixed = _split_multiwait_bir(nc.to_json_bytes())
        nc.to_json_bytes = lambda: fixed
        _NC_CACHE = nc
    return _NC_CACHE


def run(specific_features, trace=False, **trace_kw):
    """Run the device kernel; returns (per-core raw outputs, BassKernelResults)."""
    xs = np.asarray(specific_features, dtype=np.float32)
    assert xs.shape == (B, D), xs.shape
    shards = [
        np.ascontiguousarray(xs[c * ROWS_PER_CORE : (c + 1) * ROWS_PER_CORE])
        for c in range(N_CORES)
    ]
    in_maps = [{"x": s} for s in shards]
    nc = _get_nc()
    res = run_bass_kernel_spmd(
        nc, in_maps, list(range(N_CORES)), trace=trace, **trace_kw
    )
    outs = [r["out"] for r in res.results]
    return outs, res


def postprocess(outs):
    """Finish the loss from per-core [TILES, P, 10] raw dot/norm tensors."""
    # f32-faithful diagonal term: sim_ii == 1 always clips to CLAMP_MAX.
    diag_term = float(-np.log(np.float32(1.0) - np.float32(CLAMP_MAX)))
    total = 0.0
    ii = [p[0] for p in ALL_PAIRS]
    jj = [p[1] for p in ALL_PAIRS]
    for arr in outs:
        a = np.asarray(arr, dtype=np.float64)  # [TILES, P, 10]
        dots = a[..., 0:6]
        n2 = a[..., 6:10]
        norms = np.maximum(np.sqrt(n2), NORM_EPS)
        sim = dots / (norms[..., ii] * norms[..., jj])
        sim = np.clip(sim, CLAMP_MIN, CLAMP_MAX)
        # each unordered pair appears twice in the symmetric 4x4 matrix
        total += 2.0 * np.sum(-np.log1p(-sim))
    total += (B // V) * V * diag_term
    return np.float32(total / (V * V))


def kernel(specific_features):
    outs, _ = run(specific_features, trace=False)
    return postprocess(outs)


if __name__ == "__main__":
    x = np.random.default_rng(0).standard_normal((B, D)).astype(np.float32)
    print(kernel(x))
